# revision 25
# baseline (speedup 1.0000x reference)
"""Trainium2 Bass kernel: causal multi-head attention block (B=2, S=2048, D=2048, H=16).

Sharding: 8 cores = 2 (batch) x 4 (head-groups of 4 heads). Each core computes
its batch's attention output restricted to its 4 heads plus the corresponding
partial out-projection; the host sums the 4 head-group partials per batch and
adds the (o_b + o_w @ v_b) bias vector (valid because softmax rows sum to 1).
The k-bias is dropped entirely: softmax over keys is invariant to per-query
logit shifts, so only (q + bq) . k survives.

Fused single-pipeline schedule (final): projections for q-block J+1 and
out-projections for earlier blocks are generator streams interleaved as PE
"filler" into the attention phase of q-block J, so the PE never idles on the
softmax chain and the HAM clock monitor never re-throttles mid-kernel.
Filler assignment: B0<-A1, B1<-A2, B2<-A3(qk), B3<-A3(v)+C0+C1+C2, tail C3
(leftover C matmuls interleave into C3 to cover the last softmax tail).
Softmax denominators via a single all-ones matmul (ones.T @ pacc = reduce +
broadcast in one 213ns PE op, replacing a 3.5us gpsimd partition reduce).
Block 0 runs k-outer with 8 psum accumulators so the PE keeps pace with the
initial DMA; 14 warm-up matmuls on memset data keep the clock ramp alive
during the DMA lead-in. Weights/x are DMA'd as k-pair chunks (2KB partition
lines), with the first chunk split so the first matmul waits on only 384KB.
All matmuls bf16 (fp8 DoubleRow was evaluated offline: exceeds the 2e-2
tolerance), x read from HBM once, x block J+1 prefetched a block ahead.
"""

import sys

sys.path.insert(0, "/opt/trn_rl_repo")

import numpy as np
import ml_dtypes
import concourse.bacc as bacc
import concourse.tile as tile
from concourse import mybir
from concourse.bass_utils import run_bass_kernel_spmd

F32 = mybir.dt.float32
BF16 = mybir.dt.bfloat16
BFNP = ml_dtypes.bfloat16

B, S, D, H, HD = 2, 2048, 2048, 16, 128
SCALE = 1.0 / (HD**0.5)
HL = 4  # heads per core
DL = HL * HD  # 512: local head dims per core
NK = D // HD  # 16 contraction k-tiles
NJ = S // DL  # 4 blocks of 512 along sequence
NEG = -1.0e30
LAG = 2  # attnV trails its score matmul by this many iterations

_CACHE = {}


def _build():
    nc = bacc.Bacc("TRN2", target_bir_lowering=False, debug=False)
    ExpF = mybir.ActivationFunctionType.Exp
    IdF = mybir.ActivationFunctionType.Identity

    # weights/x stored as k-pair chunks: 2KB per partition line for DMA efficiency
    NP = NK // 2
    xt_d = nc.declare_dram_parameter("xt", [NJ * NP, HD, 2 * DL], BF16, isOutput=False)
    wq_d = nc.declare_dram_parameter("wq", [NP, HD, 2 * DL], BF16, isOutput=False)
    wk_d = nc.declare_dram_parameter("wk", [NP, HD, 2 * DL], BF16, isOutput=False)
    wv_d = nc.declare_dram_parameter("wv", [NP, HD, 2 * DL], BF16, isOutput=False)
    wo_d = nc.declare_dram_parameter("wo", [HL, HD, D], BF16, isOutput=False)
    bq_d = nc.declare_dram_parameter("bq", [HD, HL], F32, isOutput=False)
    mask_d = nc.declare_dram_parameter("maskT", [HD, HD], F32, isOutput=False)
    out_d = nc.declare_dram_parameter("out", [S, D], BF16, isOutput=True)

    lowprec = lambda: nc.allow_low_precision(reason="bf16 tiles")

    with tile.TileContext(nc) as tc:
        with (
            tc.tile_pool(name="const", bufs=1) as constp,
            tc.tile_pool(name="wts", bufs=1) as wts,
            tc.tile_pool(name="qk", bufs=1) as qkp,
            tc.tile_pool(name="vres", bufs=1) as vp,
            tc.tile_pool(name="xc", bufs=2) as xcp,
            tc.tile_pool(name="ptt", bufs=6) as pttp,
            tc.tile_pool(name="pacc", bufs=3) as paccp,
            tc.tile_pool(name="rec", bufs=2) as recp,
            tc.tile_pool(name="attn", bufs=4) as attnp,
            tc.tile_pool(name="ob", bufs=3) as obp,
            tc.tile_pool(name="pa", bufs=2, space="PSUM") as pap,
            tc.tile_pool(name="scp", bufs=2, space="PSUM") as pscp,
            tc.tile_pool(name="apsp", bufs=2, space="PSUM") as papsp,
            tc.tile_pool(name="opp", bufs=2, space="PSUM") as popp,
        ):
            # --- resident weights / constants ---
            wq_sb = wts.tile([HD, NK * DL], BF16, tag="wq")
            wk_sb = wts.tile([HD, NK * DL], BF16, tag="wk")
            wv_sb = wts.tile([HD, NK * DL], BF16, tag="wv")
            bq_sb = constp.tile([HD, HL], F32, tag="bq")
            mask_sb = constp.tile([HD, HD], F32, tag="mask")

            QT = [qkp.tile([HD, S], BF16, tag=f"qt{h}", name=f"qt{h}") for h in range(HL)]
            KT = [qkp.tile([HD, S], BF16, tag=f"kt{h}", name=f"kt{h}") for h in range(HL)]
            V = [vp.tile([HD, DL], BF16, tag=f"v{t}", name=f"v{t}") for t in range(S // HD)]
            # all-ones square: ones.T @ pacc = per-query key-sums replicated on
            # every partition (reduce + broadcast in a single 213ns matmul)
            ones_sb = constp.tile([HD, HD], BF16, tag="ones")
            wsrc_sb = constp.tile([HD, DL], BF16, tag="wsrc")
            nc.vector.memset(ones_sb[:], 1.0)
            nc.vector.memset(wsrc_sb[:], 0.125)

            x_blocks = [None] * NJ

            def load_x_pair(J, t):
                xp = xcp.tile([HD, 2 * DL], BF16, tag=f"x{t}", name=f"x{J}_{t}")
                nc.sync.dma_start(xp[:], xt_d[J * NP + t])
                return xp

            def load_x_block(J):
                x_blocks[J] = [load_x_pair(J, t) for t in range(NP)]

            def xtile(J, k):
                # view of k-tile k inside its pair chunk
                return x_blocks[J][k // 2][:, (k % 2) * DL : (k % 2 + 1) * DL]

            # --- initial DMAs: (wq_t, wk_t, x0_t) pair-triplets, then wv, x1, wo ---
            first_x = []
            for t in range(NP):
                sl_w = slice(2 * DL * t, 2 * DL * (t + 1))
                if t == 0:
                    # split the first chunks in half so the very first matmul
                    # only waits on 384KB instead of 768KB
                    nc.sync.dma_start(wq_sb[:, 0:DL], wq_d[0][:, 0:DL])
                    nc.sync.dma_start(wk_sb[:, 0:DL], wk_d[0][:, 0:DL])
                    xp0 = xcp.tile([HD, 2 * DL], BF16, tag="x0", name="x0_0")
                    nc.sync.dma_start(xp0[:, 0:DL], xt_d[0][:, 0:DL])
                    nc.sync.dma_start(wq_sb[:, DL : 2 * DL], wq_d[0][:, DL : 2 * DL])
                    nc.sync.dma_start(wk_sb[:, DL : 2 * DL], wk_d[0][:, DL : 2 * DL])
                    nc.sync.dma_start(xp0[:, DL : 2 * DL], xt_d[0][:, DL : 2 * DL])
                    first_x.append(xp0)
                    continue
                nc.sync.dma_start(wq_sb[:, sl_w], wq_d[t])
                nc.sync.dma_start(wk_sb[:, sl_w], wk_d[t])
                first_x.append(load_x_pair(0, t))
                if t == 1:
                    nc.sync.dma_start(bq_sb[:], bq_d[:, :])
                    nc.sync.dma_start(mask_sb[:], mask_d[:, :])
            x_blocks[0] = first_x
            for t in range(NP):
                sl_w = slice(2 * DL * t, 2 * DL * (t + 1))
                nc.sync.dma_start(wv_sb[:, sl_w], wv_d[t])

            wo_sb = []
            for dh in range(HL):
                w = wts.tile([HD, D], BF16, tag=f"wo{dh}", name=f"wo{dh}")
                wo_sb.append(w)

            # ---------- projection stream for block J (yields per PE matmul) ----------
            def a_stream(J, parts=("q", "k", "v")):
                sl_s = slice(DL * J, DL * (J + 1))
                for h in range(HL):
                    if "q" in parts:
                        qp = pap.tile([HD, DL], F32, tag="pa", name=f"qp{J}_{h}")
                        for k in range(NK):
                            sl_wh = slice(DL * k + HD * h, DL * k + HD * (h + 1))
                            nc.tensor.matmul(
                                qp[:], wq_sb[:, sl_wh], xtile(J, k),
                                start=(k == 0), stop=(k == NK - 1),
                            )
                            yield
                        with lowprec():
                            nc.scalar.activation(
                                QT[h][:, sl_s], qp[:], IdF, bias=bq_sb[:, h : h + 1]
                            )
                    if "k" in parts:
                        kp = pap.tile([HD, DL], F32, tag="pa", name=f"kp{J}_{h}")
                        for k in range(NK):
                            sl_wh = slice(DL * k + HD * h, DL * k + HD * (h + 1))
                            nc.tensor.matmul(
                                kp[:], wk_sb[:, sl_wh], xtile(J, k),
                                start=(k == 0), stop=(k == NK - 1),
                            )
                            yield
                        with lowprec():
                            nc.vector.tensor_copy(KT[h][:, sl_s], kp[:])
                if "v" in parts:
                    for t in range(4):
                        vp_ = pap.tile([HD, DL], F32, tag="pa", name=f"vp{J}_{t}")
                        for k in range(NK):
                            sl_wk = slice(DL * k, DL * (k + 1))
                            nc.tensor.matmul(
                                vp_[:], xtile(J, k)[:, HD * t : HD * (t + 1)],
                                wv_sb[:, sl_wk],
                                start=(k == 0), stop=(k == NK - 1),
                            )
                            yield
                        with lowprec():
                            nc.vector.tensor_copy(V[4 * J + t][:], vp_[:])

            # ---------- block 0: k-outer so PE keeps pace with the initial DMA ----------
            def a_block0():
                sl_s = slice(0, DL)
                qacc = [
                    pap.tile([HD, DL], F32, tag="pa", name=f"q0acc{h}")
                    for h in range(2)
                ] + [
                    pscp.tile([HD, DL], F32, tag="scp", name=f"q0acc{h}")
                    for h in range(2, HL)
                ]
                kacc = [
                    papsp.tile([HD, DL], F32, tag="aps", name=f"k0acc{h}")
                    for h in range(2)
                ] + [
                    popp.tile([HD, DL], F32, tag="op", name=f"k0acc{h}")
                    for h in range(2, HL)
                ]
                for k in range(NK):
                    for h in range(HL):
                        sl_wh = slice(DL * k + HD * h, DL * k + HD * (h + 1))
                        nc.tensor.matmul(
                            qacc[h][:], wq_sb[:, sl_wh], xtile(0, k),
                            start=(k == 0), stop=(k == NK - 1),
                        )
                        nc.tensor.matmul(
                            kacc[h][:], wk_sb[:, sl_wh], xtile(0, k),
                            start=(k == 0), stop=(k == NK - 1),
                        )
                with lowprec():
                    for h in range(HL):
                        nc.scalar.activation(
                            QT[h][:, sl_s], qacc[h][:], IdF, bias=bq_sb[:, h : h + 1]
                        )
                        nc.vector.tensor_copy(KT[h][:, sl_s], kacc[h][:])
                # V accumulators reuse the earliest-drained QK slots (q0,k0,q1,k1)
                vacc = [
                    pap.tile([HD, DL], F32, tag="pa", name="v0acc0"),
                    papsp.tile([HD, DL], F32, tag="aps", name="v0acc1"),
                    pap.tile([HD, DL], F32, tag="pa", name="v0acc2"),
                    papsp.tile([HD, DL], F32, tag="aps", name="v0acc3"),
                ]
                for k in range(NK):
                    sl_wk = slice(DL * k, DL * (k + 1))
                    for t in range(4):
                        nc.tensor.matmul(
                            vacc[t][:], xtile(0, k)[:, HD * t : HD * (t + 1)],
                            wv_sb[:, sl_wk],
                            start=(k == 0), stop=(k == NK - 1),
                        )
                with lowprec():
                    for t in range(4):
                        nc.vector.tensor_copy(V[t][:], vacc[t][:])

            # ---------- out-projection stream for block Jc ----------
            attn_t = [[None] * HL for _ in range(NJ)]

            def c_stream(Jc):
                at = attn_t[Jc]
                for c in range(4):
                    ob = obp.tile([HD, D], BF16, tag="ob", name=f"ob{Jc}_{c}")
                    sl_c = slice(HD * c, HD * (c + 1))
                    st = 4 * Jc + c
                    rows = slice(HD * st, HD * (st + 1))
                    for nb in range(4):
                        sl_n = slice(DL * nb, DL * (nb + 1))
                        op = popp.tile([HD, DL], F32, tag="op", name=f"op{Jc}_{c}_{nb}")
                        for dh in range(HL):
                            nc.tensor.matmul(
                                op[:], at[dh][:, sl_c], wo_sb[dh][:, sl_n],
                                start=(dh == 0), stop=(dh == HL - 1),
                            )
                            yield
                        with lowprec():
                            nc.vector.tensor_copy(ob[:, sl_n], op[:])
                        if nb % 2 == 1:
                            sl_h2 = slice(DL * (nb - 1), DL * (nb + 1))
                            nc.sync.dma_start(out_d[rows, sl_h2], ob[:, sl_h2])

            # ---------- attention block J with PE filler streams ----------
            def b_block(J, fillers, F):
                def pull(n):
                    while n > 0 and fillers:
                        try:
                            next(fillers[0][0])
                            n -= 1
                        except StopIteration:
                            fillers.pop(0)

                nkt = 4 * (J + 1)
                for h in range(HL):
                    Fh = F[h] if isinstance(F, (list, tuple)) else F
                    sl_h = slice(HD * h, HD * (h + 1))
                    aps = papsp.tile([HD, DL], F32, tag="aps", name=f"aps{J}_{h}")
                    pacc = paccp.tile([HD, DL], BF16, tag="pacc")
                    pend = []
                    for i in range(nkt):
                        qlo = HD * (i - 4 * J) if i >= 4 * J else 0
                        cs = slice(qlo, DL)
                        qs = slice(DL * J + qlo, DL * (J + 1))
                        scp = pscp.tile([HD, DL], F32, tag="scp", name=f"scp{J}_{h}_{i}")
                        nc.tensor.matmul(
                            scp[:, cs], KT[h][:, HD * i : HD * (i + 1)], QT[h][:, qs],
                            start=True, stop=True,
                        )
                        if i >= 4 * J:
                            dsl = slice(qlo, qlo + HD)
                            nc.vector.tensor_add(scp[:, dsl], scp[:, dsl], mask_sb[:])
                        ptt = pttp.tile([HD, DL], BF16, tag="pt")
                        with lowprec():
                            nc.scalar.activation(ptt[:, cs], scp[:, cs], ExpF)
                            if i == 0:
                                nc.vector.tensor_copy(pacc[:], ptt[:])
                            else:
                                nc.vector.tensor_add(pacc[:, cs], pacc[:, cs], ptt[:, cs])
                        pend.append((i, cs, ptt))
                        if len(pend) > LAG:
                            ip, csp, pt = pend.pop(0)
                            nc.tensor.matmul(
                                aps[:, csp], V[ip][:, sl_h], pt[:, csp],
                                start=(ip == 0), stop=False,
                            )
                        pull(Fh)
                    while pend:
                        ip, csp, pt = pend.pop(0)
                        nc.tensor.matmul(
                            aps[:, csp], V[ip][:, sl_h], pt[:, csp],
                            start=(ip == 0), stop=(not pend),
                        )
                        pull(2)
                    dps = pscp.tile([HD, DL], F32, tag="scp", name=f"dps{J}_{h}")
                    nc.tensor.matmul(dps[:], ones_sb[:], pacc[:], start=True, stop=True)
                    rec = recp.tile([HD, DL], F32, tag="rec")
                    nc.vector.reciprocal_approx_fast(rec[:], dps[:])
                    at = attnp.tile([HD, DL], BF16, tag=f"at{h}", name=f"at{J}_{h}")
                    with lowprec():
                        nc.vector.tensor_mul(at[:], aps[:], rec[:])
                    attn_t[J][h] = at
                    pull(Fh)
                # drain must-finish fillers (projections the next block depends on)
                rest = []
                for gen, must in fillers:
                    if must:
                        for _ in gen:
                            pass
                    else:
                        rest.append([gen, must])
                return rest

            # ---------- main schedule ----------
            # PE warm-up on memset data during the initial DMA window: keeps
            # the HAM activity monitor ramping while weights stream in
            warm = pscp.tile([HD, DL], F32, tag="scp", name="warm")
            for _ in range(14):
                nc.tensor.matmul(warm[:], ones_sb[:], wsrc_sb[:], start=True, stop=True)
            a_block0()
            load_x_block(1)
            for dh in range(HL):
                nc.sync.dma_start(wo_sb[dh][:], wo_d[dh])

            left = b_block(0, [[a_stream(1), True]], F=12)
            load_x_block(2)
            left = b_block(1, left + [[a_stream(2), True]], F=6)
            load_x_block(3)
            left = b_block(2, left + [[a_stream(3, ("q", "k")), True]], F=3)
            left = b_block(
                3,
                left
                + [
                    [a_stream(3, ("v",)), True],
                    [c_stream(0), False],
                    [c_stream(1), False],
                    [c_stream(2), False],
                ],
                F=[4, 3, 3, 2],
            )
            # tail: out-projection of the last block, with leftover filler
            # (late C(2) matmuls) covering the last head's softmax-tail chain
            def pull_left(n):
                while n > 0 and left:
                    try:
                        next(left[0][0])
                        n -= 1
                    except StopIteration:
                        left.pop(0)

            for _ in c_stream(NJ - 1):
                pull_left(1)
            for gen, _ in left:
                for _ in gen:
                    pass

    nc.compile()
    return nc


def _prep_in_maps(x, q_w, q_b, k_w, k_b, v_w, v_b, o_w, o_b):
    mask = np.where(
        np.arange(HD)[:, None] > np.arange(HD)[None, :], np.float32(NEG), np.float32(0)
    ).astype(np.float32)
    in_maps = []
    for c in range(8):
        b, hg = divmod(c, 4)
        ds = slice(DL * hg, DL * (hg + 1))
        NP = NK // 2

        def pair_w(w):  # [NK, HD, DL] -> [NP, HD, 2*DL] (k-pairs per partition line)
            return np.ascontiguousarray(
                w.reshape(NP, 2, HD, DL).transpose(0, 2, 1, 3).reshape(NP, HD, 2 * DL)
            )

        xT = np.ascontiguousarray(x[b].T.astype(BFNP))  # [D, S]
        xt = np.ascontiguousarray(
            xT.reshape(NK, HD, NJ, DL)
            .transpose(2, 0, 1, 3)
            .reshape(NJ, NP, 2, HD, DL)
            .transpose(0, 1, 3, 2, 4)
            .reshape(NJ * NP, HD, 2 * DL)
        )
        wq = pair_w((q_w[ds].T * SCALE).astype(BFNP).reshape(NK, HD, DL))
        wk = pair_w(k_w[ds].T.astype(BFNP).reshape(NK, HD, DL))
        wv = pair_w(v_w[ds].T.astype(BFNP).reshape(NK, HD, DL))
        wo = np.ascontiguousarray(o_w[:, ds].T.astype(BFNP).reshape(HL, HD, D))
        in_maps.append(
            {
                "xt": xt,
                "wq": wq,
                "wk": wk,
                "wv": wv,
                "wo": wo,
                "bq": np.ascontiguousarray((q_b[ds] * SCALE).reshape(HL, HD).T),
                "maskT": mask,
            }
        )
    return in_maps


def kernel(x, q_w, q_b, k_w, k_b, v_w, v_b, o_w, o_b, _trace=False, _trace_kwargs=None):
    x = np.asarray(x, np.float32)
    args = [np.asarray(a, np.float32) for a in (q_w, q_b, k_w, k_b, v_w, v_b, o_w, o_b)]
    q_w, q_b, k_w, k_b, v_w, v_b, o_w, o_b = args

    if "nc" not in _CACHE:
        _CACHE["nc"] = _build()
    nc = _CACHE["nc"]

    in_maps = _prep_in_maps(x, q_w, q_b, k_w, k_b, v_w, v_b, o_w, o_b)
    res = run_bass_kernel_spmd(
        nc, in_maps, list(range(8)), trace=_trace, **(_trace_kwargs or {})
    )
    _CACHE["last_result"] = res

    bias_vec = (o_w @ v_b + o_b).astype(np.float32)
    out = np.empty((B, S, D), np.float32)
    for b in range(B):
        acc = res.results[4 * b]["out"].astype(np.float32)
        for hg in range(1, 4):
            acc = acc + res.results[4 * b + hg]["out"].astype(np.float32)
        out[b] = acc + bias_vec
    return out
